# revision 5
# baseline (speedup 1.0000x reference)
"""Trainium2 Bass kernel for CSI2PointCloudLoss (chamfer + feature-transform reg).

Full inputs in, full (scalar) output out. Internally: data-parallel over the
batch dimension across 8 NeuronCores (2 batches per core).

Math per batch b:
  chamfer_b = mean_n min_m dist(p_n, t_m) + mean_m min_n dist(p_n, t_m)
  reg_b     = ||T @ T^T - I||_F
  loss      = mean_b chamfer_b + 0.1 * mean_b reg_b

Device strategy per batch (log-sum-exp formulation + DVE direct path):
  - d2[n, m] = |p_n|^2 - 2 p_n.t_m + |t_m|^2 on the TensorEngine as K=13
    matmuls; rows hold split-bf16 (hi/lo) coordinates + norm terms so d2 is
    accurate to ~1e-4 absolute at bf16 PE speed. Each PSUM tile holds ONE
    n-tile x 4 m-groups (quadrant i -> m-group 4h+i).
  - exp path (most tiles): ScalarE computes e = exp(-LAMBDA*d2) PSUM->SBUF
    bf16; accum_out delivers rowsum[n] = sum_m e for free (soft row-min,
    recovered host-side as -ln(rowsum)/LAMBDA). DVE accumulates
    colmax[m] = max_n e (exact col-min since max of exp = exp of -min).
  - direct path (DIRECT_T tiles): ScalarE is the bottleneck, so a few tiles
    bypass it: DVE copy-casts PSUM->bf16, min-folds a row tree (exact row
    min), and min-accumulates a separate d2 col accumulator.
  - Both col accumulators get a partition-axis finish via PE transpose +
    DVE reduce (deferred finale overlapped with the next batch's stream).
    Host combines: colmin[m] = min(direct_colmin, -ln(exp_colmax)/LAMBDA).
  - Underflowed rows/cols (outliers with d2min > ~88/LAMBDA) are clamped
    host-side; contribution ~1e-4 relative.
  - reg: gram via 3 accumulating bf16-split matmuls; (gram - I) squared and
    row-summed on ScalarE (Square is in the same act table set as Exp);
    final sqrt on host.
"""

import numpy as np
import ml_dtypes

import concourse.bass as bass
from concourse import bacc
import concourse.mybir as mybir
import concourse.tile as tile
from concourse.bass_utils import run_bass_kernel_spmd
from concourse.masks import make_identity

N_CORES = 8
B, N, M, K = 16, 4096, 4096, 64
BPC = B // N_CORES  # batches per core
NT = N // 128  # 32 n-tiles
KROWS = 13  # lhsT/rhs contraction rows (fits one 32-row PE group)
LAMBDA = 512.0
DIRECT_T = (5, 16, 27)  # n-tiles handled by the DVE direct path
ND = len(DIRECT_T)
# per-batch staging layout:
#   rowstage [128, 64]  : exp-path row sums (cols 2t+h; direct tiles unused)
#   colstage [128, 33+2*ND]: 32 exp colmax | reg | 2*ND direct row d2 mins
#   (direct colmin d2 [128, 32] appended after)
CSW = 33 + 2 * ND
BLK = 64 + CSW + 32
OUT_W = BPC * BLK

F32 = mybir.dt.float32
BF16 = mybir.dt.bfloat16
BF16_NP = ml_dtypes.bfloat16

LAST_RESULTS = None  # BassKernelResults of the most recent run (for profiling)
_PROGRAM = None


def _kernel_body(ctx, tc, oo, pp, gg, tt):
    nc = tc.nc
    AL = mybir.AluOpType
    AX = mybir.AxisListType
    AF = mybir.ActivationFunctionType

    singles = ctx.enter_context(tc.tile_pool(name="singles", bufs=1))
    packs = ctx.enter_context(tc.tile_pool(name="packs", bufs=2))
    psum = ctx.enter_context(tc.tile_pool(name="psum", bufs=2, space="PSUM"))
    casts = ctx.enter_context(tc.tile_pool(name="casts", bufs=6))
    acc = ctx.enter_context(tc.tile_pool(name="acc", bufs=2))
    small = ctx.enter_context(tc.tile_pool(name="small", bufs=4))

    identb = singles.tile([128, 128], BF16, name="identb")
    make_identity(nc, identb[:])
    identf = singles.tile([64, 64], F32, name="identf")
    make_identity(nc, identf[:])

    INF = float(np.inf)
    pending_finale = []

    for b in range(BPC):
        # --- load packed point rows, replicated at partition bases 0/32/64/96
        ppack = packs.tile([128, N], BF16, tag="ppack", name="ppack")
        gpack = packs.tile([128, M], BF16, tag="gpack", name="gpack")
        for i in range(4):
            nc.sync.dma_start(ppack[32 * i : 32 * i + KROWS, :], pp[b])
            nc.sync.dma_start(gpack[32 * i : 32 * i + KROWS, :], gg[b])

        colacc = acc.tile([128, 2, 2048], BF16, tag="colacc", name="colacc")
        nc.gpsimd.memset(colacc[:], 0.0)
        colacc2 = acc.tile([128, 2, 2048], BF16, tag="colacc2", name="colacc2")
        nc.gpsimd.memset(colacc2[:], INF)
        rowstage = acc.tile([128, 64], F32, tag="rowstage", name="rowstage")
        nc.scalar.memzero(rowstage[:])
        colstage = acc.tile([128, CSW], F32, tag="colstage", name="colstage")

        for t in range(NT):
            direct = t in DIRECT_T
            for h in range(2):
                ps = psum.tile([128, 2048], F32, tag="ps", name="ps")
                for i in range(4):
                    g = 4 * h + i
                    nc.tensor.matmul(
                        ps[:, 512 * i : 512 * (i + 1)],
                        ppack[32 * i : 32 * i + KROWS, 128 * t : 128 * (t + 1)],
                        gpack[32 * i : 32 * i + KROWS, 512 * g : 512 * (g + 1)],
                        start=True,
                        stop=True,
                        tile_position=(32 * i, 0),
                    )
                cb = casts.tile([128, 2048], BF16, tag="cb", name="cb")
                if direct:
                    k = DIRECT_T.index(t)
                    nc.vector.tensor_copy(cb[:], ps[:])
                    nc.vector.tensor_tensor(
                        colacc2[:, h, :], cb[:], colacc2[:, h, :], AL.min
                    )
                    rtmp = small.tile([128, 1024], BF16, tag="rtmp", name="rtmp")
                    nc.vector.tensor_tensor(
                        rtmp[:], cb[:, 0:1024], cb[:, 1024:2048], AL.min
                    )
                    nc.vector.tensor_tensor(
                        rtmp[:, 0:512], rtmp[:, 0:512], rtmp[:, 512:1024], AL.min
                    )
                    nc.vector.tensor_reduce(
                        colstage[:, 33 + 2 * k + h : 34 + 2 * k + h],
                        rtmp[:, 0:512],
                        axis=AX.X,
                        op=AL.min,
                    )
                else:
                    nc.scalar.activation(
                        cb[:],
                        ps[:],
                        AF.Exp,
                        scale=-LAMBDA,
                        accum_out=rowstage[:, 2 * t + h : 2 * t + h + 1],
                    )
                    nc.vector.tensor_tensor(
                        colacc[:, h, :], cb[:], colacc[:, h, :], AL.max
                    )
            if t == 0 and pending_finale:
                pending_finale.pop()()

        def _finale(b=b, colacc=colacc, colacc2=colacc2, rowstage=rowstage,
                    colstage=colstage):
            _emit_finale(nc, tc, small, psum, oo, rowstage, colstage, identb,
                         identf, tt, b, colacc, colacc2)

        pending_finale.append(_finale)

    while pending_finale:
        pending_finale.pop()()


def _emit_finale(nc, tc, small, psum, oo, rowstage, colstage, identb, identf,
                 tt, b, colacc, colacc2):
    AL = mybir.AluOpType
    AX = mybir.AxisListType
    AF = mybir.ActivationFunctionType

    # --- col side: partition-axis reduce via PE transpose (bf16) + DVE reduce
    dcol = small.tile([128, 32], F32, tag="dcol", name="dcol")
    for half in range(2):
        pst = psum.tile([128, 16, 128], BF16, tag="ps", name="pst")
        for k in range(16):
            nc.tensor.transpose(
                pst[:, k, :],
                colacc[:, half, 128 * k : 128 * (k + 1)],
                identb[:],
            )
        nc.vector.tensor_reduce(
            colstage[:, 16 * half : 16 * (half + 1)],
            pst[:],
            axis=AX.X,
            op=AL.max,
        )
        pst2 = psum.tile([128, 16, 128], BF16, tag="ps", name="pst2")
        for k in range(16):
            nc.tensor.transpose(
                pst2[:, k, :],
                colacc2[:, half, 128 * k : 128 * (k + 1)],
                identb[:],
            )
        nc.vector.tensor_reduce(
            dcol[:, 16 * half : 16 * (half + 1)],
            pst2[:],
            axis=AX.X,
            op=AL.min,
        )

    # --- regularizer: gram = T @ T^T via split-bf16 (3 accumulating MMs)
    tA = small.tile([128, K], BF16, tag="tA", name="tA")  # [hi; lo]
    tB = small.tile([64, K], BF16, tag="tB", name="tB")  # lo at parts 0-63
    nc.sync.dma_start(tA[:], tt[b])
    nc.sync.dma_start(tB[:], tt[b, 64:128])
    pg = psum.tile([64, 64], F32, tag="ps", name="pg")
    hi = tA[0:64, :]
    lo = tB[0:64, :]
    nc.tensor.matmul(pg[:], hi, hi, start=True, stop=False)
    nc.tensor.matmul(pg[:], lo, hi, start=False, stop=False)
    nc.tensor.matmul(pg[:], hi, lo, start=False, stop=True)
    nc.vector.tensor_tensor(pg[:], pg[:], identf[:], AL.subtract)
    gtrash = small.tile([64, K], F32, tag="gtrash", name="gtrash")
    nc.scalar.activation(
        gtrash[:],
        pg[:],
        AF.Square,
        accum_out=colstage[0:64, 32:33],
    )
    nc.sync.dma_start(oo[:, BLK * b : BLK * b + 64], rowstage[:])
    nc.sync.dma_start(oo[:, BLK * b + 64 : BLK * b + 64 + CSW], colstage[:])
    nc.sync.dma_start(oo[:, BLK * b + 64 + CSW : BLK * (b + 1)], dcol[:])


def _build_program():
    from contextlib import ExitStack

    nc = bacc.Bacc(
        "TRN2", target_bir_lowering=False, debug=False, num_devices=N_CORES
    )
    pp = nc.dram_tensor("pp", [BPC, KROWS, N], BF16, kind="ExternalInput").ap()
    gg = nc.dram_tensor("gg", [BPC, KROWS, M], BF16, kind="ExternalInput").ap()
    tt = nc.dram_tensor("tt", [BPC, 128, K], BF16, kind="ExternalInput").ap()
    oo = nc.dram_tensor("oo", [128, OUT_W], F32, kind="ExternalOutput").ap()
    with tile.TileContext(nc) as tc:
        with ExitStack() as ctx:
            _kernel_body(ctx, tc, oo, pp, gg, tt)
    nc.finalize()
    return nc


def _get_program():
    global _PROGRAM
    if _PROGRAM is None:
        _PROGRAM = _build_program()
    return _PROGRAM


def _split(x):
    """f32 -> (hi, lo) bf16 split with hi + lo ~= x to ~2^-17 rel."""
    hi = x.astype(BF16_NP)
    lo = (x - hi.astype(np.float32)).astype(BF16_NP)
    return hi, lo


def _pack_inputs(predicted_points, gt_points, trans_feat):
    """Build per-core input maps for the device program."""
    p = np.asarray(predicted_points, dtype=np.float32)
    t = np.asarray(gt_points, dtype=np.float32)
    tr = np.asarray(trans_feat, dtype=np.float32)

    ph, pl = _split(p)  # [B, N, 3]
    th, tl = _split(t)  # [B, M, 3]
    p_acc = ph.astype(np.float32) + pl.astype(np.float32)
    t_acc = th.astype(np.float32) + tl.astype(np.float32)
    pn2 = np.sum(p_acc * p_acc, axis=-1)  # [B, N]
    tn2 = np.sum(t_acc * t_acc, axis=-1)  # [B, M]
    pn2h, pn2l = _split(pn2)
    tn2h, tn2l = _split(tn2)

    ones = np.ones((B, N), dtype=BF16_NP)

    # pred-side lhsT rows [B, 13, N]
    pp_rows = np.stack(
        [
            ph[..., 0], ph[..., 1], ph[..., 2],
            pl[..., 0], pl[..., 1], pl[..., 2],
            ph[..., 0], ph[..., 1], ph[..., 2],
            pn2h, pn2l, ones, ones,
        ],
        axis=1,
    )
    nth = (-2.0 * th.astype(np.float32)).astype(BF16_NP)
    ntl = (-2.0 * tl.astype(np.float32)).astype(BF16_NP)
    gg_rows = np.stack(
        [
            nth[..., 0], nth[..., 1], nth[..., 2],
            nth[..., 0], nth[..., 1], nth[..., 2],
            ntl[..., 0], ntl[..., 1], ntl[..., 2],
            ones, ones, tn2h, tn2l,
        ],
        axis=1,
    )
    trh, trl = _split(tr)  # [B, 64, 64]
    tt_rows = np.concatenate([trh, trl], axis=1)  # [B, 128, 64]

    in_maps = []
    for c in range(N_CORES):
        sl = slice(c * BPC, (c + 1) * BPC)
        in_maps.append(
            {
                "pp": np.ascontiguousarray(pp_rows[sl]),
                "gg": np.ascontiguousarray(gg_rows[sl]),
                "tt": np.ascontiguousarray(tt_rows[sl]),
            }
        )
    return in_maps


def kernel(predicted_points, ground_truth_points, trans_feat):
    global LAST_RESULTS
    nc = _get_program()
    in_maps = _pack_inputs(predicted_points, ground_truth_points, trans_feat)
    res = run_bass_kernel_spmd(nc, in_maps, core_ids=list(range(N_CORES)))
    LAST_RESULTS = res

    # exp(-88.5) is below the smallest bf16/f32 normal; an underflowed
    # row/col means d2min > UNDERFLOW_D2 and we clamp there.
    UNDERFLOW_D2 = 88.0 / LAMBDA
    direct = np.array(DIRECT_T)
    total = 0.0
    for c in range(N_CORES):
        o = res.results[c]["oo"].astype(np.float64)  # [128, OUT_W]
        for b in range(BPC):
            blk = o[:, BLK * b : BLK * (b + 1)]
            rowsum = blk[:, 0:64:2] + blk[:, 1:64:2]  # [128, 32]
            d2row = np.where(
                rowsum > 0.0,
                -np.log(np.maximum(rowsum, 1e-300)) / LAMBDA,
                UNDERFLOW_D2,
            )
            # direct tiles: exact row mins, overwrite the (garbage) soft vals
            drow = blk[:, 97 : 97 + 2 * ND]  # [128, 2*ND]
            d2row[:, direct] = np.minimum(drow[:, 0::2], drow[:, 1::2])
            min1 = np.sqrt(np.maximum(d2row, 0.0))
            colmax = blk[:, 64:96]  # [128, 32] exp-path col max
            d2col = np.where(
                colmax > 0.0,
                -np.log(np.maximum(colmax, 1e-300)) / LAMBDA,
                UNDERFLOW_D2,
            )
            dcol = blk[:, 64 + CSW : 64 + CSW + 32]  # direct-path col d2 min
            d2col = np.minimum(d2col, dcol)
            min2 = np.sqrt(np.maximum(d2col, 0.0))
            chamfer = min1.mean() + min2.mean()
            reg = np.sqrt(blk[0:64, 96].sum())
            total += chamfer + 0.1 * reg
    return np.float32(total / B)
